# revision 19
# baseline (speedup 1.0000x reference)
"""Causal self-attention (B=4, T=2048, C=1024, H=16) on 8 trn2 cores.

Sharding: batch (4-way) x head-group (2-way).  Core i handles batch i//2 and
heads [8*(i%2), 8*(i%2)+8).  Each core computes qkv projection for its head
slice, causal attention, and a partial out-projection (contraction over its
512 att columns).  Host sums the two partials per batch.

All device compute is fp32 (matmuls as float32r).  Host pre-transposes so the
device never transposes anything:
  - xT       (C, T)      : x[b].T
  - wqk      (8,128,8*128): per m-tile of [wq_g; wk_g].T, k-tiles along free
  - wv       (C, 512)    : wv_g.T
  - wo       (512, C)    : w_out.T row-slice for this head group
  - masks    (4,128,512) : binary causal masks for the 4 diagonal offsets
Layouts on chip:
  - QT/KT  [128, 4, T]   rows = head-major (hl*64+d), T on free dim
  - V      [128, 16, 772]: per t-tile, per head pair [V_e|1|0*63|1|V_o] cols
  - attT   [128, 4, T]   rows = c_local = hl*64+d  (lhsT for out-proj)
"""

from contextlib import ExitStack

import numpy as np

import concourse.bass as bass
import concourse.mybir as mybir
import concourse.tile as tile
from concourse import bacc, bass_utils

B, T, C, H, HD = 4, 2048, 1024, 16, 64
HG = 2  # head groups (tensor-parallel dim)
HPG = H // HG  # 8 heads per group
OG = HPG * HD  # 512: local width of q/k/v slice
KT_C = C // 128  # 8 contraction tiles for the projections
NT = T // 128  # 16 t-tiles
NQ = T // 512  # 4 tq chunks
PAIR_W = 65 + 128  # v_sb cols per head pair: [V_e|1] + [0*63|1|V_o]

f32 = mybir.dt.float32
f32r = mybir.dt.float32r

TRACE = False  # test.py flips this for profiling runs
DEBUG = False  # adds intermediate dumps (qt/kt/v/attT) as extra outputs
LAST_RUN = {}

_NC_CACHE = []


def _mm(nc, out, lhsT, rhs, **kw):
    nc.tensor.matmul(out, lhsT, rhs, **kw)


def _build_nc():
    nc = bacc.Bacc(trn_type="TRN2", target_bir_lowering=False, debug=False)
    xT = nc.dram_tensor("xT", [C, T], f32r, kind="ExternalInput").ap()
    wqk = nc.dram_tensor("wqk", [8, 128, 1024], f32r, kind="ExternalInput").ap()
    wv = nc.dram_tensor("wv", [C, OG], f32r, kind="ExternalInput").ap()
    wo = nc.dram_tensor("wo", [OG, C], f32r, kind="ExternalInput").ap()
    masks = nc.dram_tensor("masks", [4, 128, 512], f32r, kind="ExternalInput").ap()
    vinit = nc.dram_tensor("vinit", [128, 4 * PAIR_W], f32r, kind="ExternalInput").ap()
    ones_in = nc.dram_tensor("ones_in", [128, 128], f32r, kind="ExternalInput").ap()
    y = nc.dram_tensor("y", [T, C], f32, kind="ExternalOutput").ap()
    dbg = None
    if DEBUG:
        dbg = {
            "qt": nc.dram_tensor("dbg_qt", [128, 4, T], f32r, kind="ExternalOutput").ap(),
            "kt": nc.dram_tensor("dbg_kt", [128, 4, T], f32r, kind="ExternalOutput").ap(),
            "v": nc.dram_tensor("dbg_v", [128, NT, 4 * PAIR_W], f32r, kind="ExternalOutput").ap(),
            "attT": nc.dram_tensor("dbg_attT", [128, 4, T], f32r, kind="ExternalOutput").ap(),
        }

    with tile.TileContext(nc) as tc:
        _body(tc, nc, xT, wqk, wv, wo, masks, vinit, ones_in, y, dbg)
    nc.compile()
    return nc


def _body(tc, nc, xT, wqk, wv, wo, masks, vinit, ones_in, y, dbg):
    exp_f = mybir.ActivationFunctionType.Exp

    # ---- persistent tensors (allocated below the per-phase pools) ----
    with tc.tile_pool(name="persist", bufs=1) as persist:
        qt = persist.tile([128, 4, T], f32r)
        kt = persist.tile([128, 4, T], f32r)
        v_sb = persist.tile([128, NT, 4 * PAIR_W], f32r)

        # init v_sb from DRAM: ones columns + inter-head zeros (memset cannot
        # write f32r); the V regions are overwritten by phase-1 copies.
        for tt in range(NT):
            nc.sync.dma_start(v_sb[:, tt, :], vinit[:])

        # ================= phase 1: qkv projections =================
        with (
            tc.tile_pool(name="wv_p", bufs=1) as wv_p,
            tc.tile_pool(name="xh_p", bufs=1) as xh_p,
            tc.tile_pool(name="wqk_p", bufs=2) as wqk_p,
            tc.tile_pool(name="p1ps", bufs=2, space="PSUM") as p1ps,
            tc.tile_pool(name="p1ps_v", bufs=2, space="PSUM") as p1ps_v,
        ):
            wv_sb = wv_p.tile([128, KT_C, OG], f32r)
            for k in range(KT_C):
                nc.sync.dma_start(wv_sb[:, k, :], wv[k * 128 : (k + 1) * 128, :])

            for half in range(2):
                t0 = half * (T // 2)
                xh = xh_p.tile([128, KT_C, T // 2], f32r, tag="xh")
                for k in range(KT_C):
                    nc.sync.dma_start(
                        xh[:, k, :], xT[k * 128 : (k + 1) * 128, t0 : t0 + T // 2]
                    )
                # Q^T / K^T : out rows o (head-major), free = t
                for m in range(8):
                    wt = wqk_p.tile([128, 1024], f32r, tag="wqk")
                    nc.sync.dma_start(wt[:], wqk[m, :, :])
                    dst = qt if m < 4 else kt
                    for n in range(2):
                        ps = p1ps.tile([128, 512], f32, tag="p1")
                        for k in range(KT_C):
                            _mm(
                                nc,
                                ps[:],
                                wt[:, k * 128 : (k + 1) * 128],
                                xh[:, k, n * 512 : (n + 1) * 512],
                                start=(k == 0),
                                stop=(k == KT_C - 1),
                            )
                        nc.scalar.copy(
                            dst[:, m % 4, t0 + n * 512 : t0 + (n + 1) * 512], ps[:]
                        )
                # V: out rows t, free = o (head-major)
                for tl in range(T // 2 // 128):
                    tt = half * 8 + tl
                    ps = p1ps_v.tile([128, 512], f32, tag="p1v")
                    for k in range(KT_C):
                        _mm(
                            nc,
                            ps[:],
                            xh[:, k, tl * 128 : (tl + 1) * 128],
                            wv_sb[:, k, :],
                            start=(k == 0),
                            stop=(k == KT_C - 1),
                        )
                    # scatter per head into v_sb blocks (even | odd)
                    src_e = ps[:].rearrange("p (h d) -> p h d", d=64)[:, 0::2, :]
                    src_o = ps[:].rearrange("p (h d) -> p h d", d=64)[:, 1::2, :]
                    dst = v_sb[:, tt, :].rearrange("p (q w) -> p q w", w=PAIR_W)
                    nc.vector.tensor_copy(dst[:, :, 0:64], src_e)
                    nc.vector.tensor_copy(dst[:, :, 129:193], src_o)

        if dbg is not None:
            for mm_ in range(4):
                nc.sync.dma_start(dbg["qt"][:, mm_, :], qt[:, mm_, :])
                nc.sync.dma_start(dbg["kt"][:, mm_, :], kt[:, mm_, :])
            for tt_ in range(NT):
                nc.sync.dma_start(dbg["v"][:, tt_, :], v_sb[:, tt_, :])

        # ================= phase 2: attention =================
        # attT opens after phase-1 pools close so the stack allocator reuses
        # their SBUF; it stays open through phase 3 (closed at the end).
        att_ctx = ExitStack()
        attp = att_ctx.enter_context(tc.tile_pool(name="attp", bufs=1))
        attT = attp.tile([128, 4, T], f32r)
        with (
            tc.tile_pool(name="mask_p", bufs=1) as mask_p,
            tc.tile_pool(name="pt_p", bufs=3) as pt_p,
            tc.tile_pool(name="recip_p", bufs=2) as recip_p,
            tc.tile_pool(name="bcast_p", bufs=2) as bcast_p,
            tc.tile_pool(name="ones_p", bufs=1) as ones_p,
            tc.tile_pool(name="st_ps", bufs=3, space="PSUM") as st_ps,
            tc.tile_pool(name="av_ps", bufs=1, space="PSUM") as av_ps,
        ):
            mk = mask_p.tile([128, 4, 512], f32r)
            for vv in range(4):
                nc.sync.dma_start(mk[:, vv, :], masks[vv, :, :])
            ones_sb = ones_p.tile([128, 128], f32r)
            nc.sync.dma_start(ones_sb[:], ones_in[:])

            def head_ctx(hl):
                """Slices/layout facts for local head hl."""
                p0 = (hl % 2) * 64
                mt = hl // 2
                qrow = slice(p0, p0 + 64)
                vb0 = (hl // 2) * PAIR_W
                if hl % 2 == 0:
                    vsl = (vb0, vb0 + 65)  # [V|1] -> rows 0..64
                    srow, arow = 64, slice(0, 64)
                else:
                    vsl = (vb0 + 65, vb0 + 193)  # [1|0*63|V] -> row 0 sums, 64..127 att
                    srow, arow = 0, slice(64, 128)
                return p0, mt, qrow, vsl, srow, arow

            # tk-outer over chunk PAIRS: each kt(tk)/v(tk) stationary load
            # feeds two matmuls (tq chunks 2jp and 2jp+1), so the PE drain
            # overlaps the next fill instead of paying (N+219) per matmul.
            for hl in range(HPG):
                p0, mt, qrow, vsl, srow, arow = head_ctx(hl)
                for jp in range(NQ // 2):
                    ca, cb = 2 * jp, 2 * jp + 1
                    tqa = slice(ca * 512, (ca + 1) * 512)
                    tqb = slice(cb * 512, (cb + 1) * 512)
                    ntk_a, ntk_b = 4 * ca + 4, 4 * cb + 4

                    pts = [None] * ntk_b

                    def emit_st(tk):
                        wide = tk < ntk_a
                        ps = st_ps.tile([128, 1024], f32, tag="st")
                        if wide:
                            _mm(nc, ps[:, 0:512],
                                kt[qrow, mt, tk * 128 : (tk + 1) * 128],
                                qt[qrow, mt, tqa], start=True, stop=True)
                            _mm(nc, ps[:, 512:1024],
                                kt[qrow, mt, tk * 128 : (tk + 1) * 128],
                                qt[qrow, mt, tqb], start=True, stop=True)
                        else:
                            _mm(nc, ps[:, 0:512],
                                kt[qrow, mt, tk * 128 : (tk + 1) * 128],
                                qt[qrow, mt, tqb], start=True, stop=True)
                        pt = pt_p.tile([128, 1024], f32r, tag="pt")
                        w = 1024 if wide else 512
                        nc.scalar.activation(pt[:, 0:w], ps[:, 0:w], exp_f, scale=0.125)
                        # causal masks on the diagonal blocks
                        if wide and tk >= 4 * ca:
                            nc.vector.tensor_mul(
                                pt[:, 0:512], pt[:, 0:512], mk[:, tk - 4 * ca, :])
                        bslot = slice(512, 1024) if wide else slice(0, 512)
                        if tk >= 4 * cb:
                            nc.vector.tensor_mul(
                                pt[:, bslot], pt[:, bslot], mk[:, tk - 4 * cb, :])
                        pts[tk] = (pt, bslot)

                    ava = av_ps.tile([128, 512], f32, tag="ava")
                    avb = av_ps.tile([128, 512], f32, tag="avb")
                    emit_st(0)
                    for tk in range(ntk_b):
                        if tk + 1 < ntk_b:
                            emit_st(tk + 1)
                        pt, bslot = pts[tk]
                        if tk < ntk_a:
                            _mm(nc, ava[0 : vsl[1] - vsl[0], :],
                                v_sb[:, tk, vsl[0] : vsl[1]], pt[:, 0:512],
                                start=(tk == 0), stop=(tk == ntk_a - 1))
                        _mm(nc, avb[0 : vsl[1] - vsl[0], :],
                            v_sb[:, tk, vsl[0] : vsl[1]], pt[:, bslot],
                            start=(tk == 0), stop=(tk == ntk_b - 1))
                        pts[tk] = None

                    # normalize + write attT for both chunks
                    for av, tq in ((ava, tqa), (avb, tqb)):
                        sums_sb = recip_p.tile([128, 512], f32r, tag="rc")
                        nc.scalar.copy(
                            sums_sb[srow : srow + 1, :], av[srow : srow + 1, :])
                        bps = st_ps.tile([128, 1024], f32, tag="st", name=f"bps_{hl}_{jp}_{tq.start}")
                        _mm(nc, bps[:, 0:512],
                            ones_sb[srow : srow + 1, :],
                            sums_sb[srow : srow + 1, :], start=True, stop=True)
                        bc = bcast_p.tile([128, 512], f32, tag="bc")
                        nc.vector.reciprocal_approx_fast(bc[:], bps[:, 0:512])
                        nc.vector.tensor_mul(
                            attT[arow, mt, tq], av[arow, :], bc[arow, :])

        if dbg is not None:
            for mm_ in range(4):
                nc.sync.dma_start(dbg["attT"][:, mm_, :], attT[:, mm_, :])

        # ================= phase 3: out-projection =================
        with (
            tc.tile_pool(name="wo_p", bufs=1) as wo_p,
            tc.tile_pool(name="yo_p", bufs=3) as yo_p,
            tc.tile_pool(name="y_ps", bufs=4, space="PSUM") as y_ps,
        ):
            wo_sb = wo_p.tile([128, 4, C], f32r)
            for k in range(4):
                nc.sync.dma_start(wo_sb[:, k, :], wo[k * 128 : (k + 1) * 128, :])
            for tt in range(NT):
                for o in range(C // 512):
                    ps = y_ps.tile([128, 512], f32, tag="y")
                    for k in range(4):
                        _mm(
                            nc,
                            ps[:],
                            attT[:, k, tt * 128 : (tt + 1) * 128],
                            wo_sb[:, k, o * 512 : (o + 1) * 512],
                            start=(k == 0),
                            stop=(k == 3),
                        )
                    yo = yo_p.tile([128, 512], f32, tag="yo")
                    nc.scalar.copy(yo[:], ps[:])
                    nc.sync.dma_start(
                        y[tt * 128 : (tt + 1) * 128, o * 512 : (o + 1) * 512], yo[:]
                    )
        att_ctx.close()


def _round_fp32r(a):
    """Round fp32 to the fp32r grid (11 mantissa bits; low 12 bits zero), RNE."""
    u = np.ascontiguousarray(a, dtype=np.float32).view(np.uint32)
    lsb = (u >> 12) & 1
    out = ((u + 0x7FF + lsb) & 0xFFFFF000).astype(np.uint32)
    return out.view(np.float32)


def _host_prep(x, w_qkv, w_out):
    xT_all = np.ascontiguousarray(x.transpose(0, 2, 1)).astype(np.float32)
    masks = np.zeros((4, 128, 512), np.float32)
    tk_l = np.arange(128)[:, None]
    tq_l = np.arange(512)[None, :]
    for vv in range(4):
        masks[vv] = (tq_l - tk_l >= vv * 128).astype(np.float32)

    per_group = []
    for g in range(HG):
        wq = w_qkv[g * OG : (g + 1) * OG]
        wk = w_qkv[C + g * OG : C + (g + 1) * OG]
        wvg = w_qkv[2 * C + g * OG : 2 * C + (g + 1) * OG]
        wqkT = np.concatenate([wq, wk], axis=0).T  # (C, 1024)
        # wqk_r[m, p, k*128+j] = wqkT[k*128+p, m*128+j]
        wqk_r = np.ascontiguousarray(
            wqkT.reshape(8, 128, 8, 128).transpose(2, 1, 0, 3).reshape(8, 128, 1024)
        ).astype(np.float32)
        wv_t = np.ascontiguousarray(wvg.T).astype(np.float32)  # (C, 512)
        wo_t = np.ascontiguousarray(w_out.T[g * OG : (g + 1) * OG]).astype(
            np.float32
        )  # (512, C)
        per_group.append((_round_fp32r(wqk_r), _round_fp32r(wv_t), _round_fp32r(wo_t)))
    vinit = np.zeros((128, 4 * PAIR_W), np.float32)
    for pr in range(4):
        vinit[:, pr * PAIR_W + 64] = 1.0  # even-head ones col
        vinit[:, pr * PAIR_W + 65] = 1.0  # odd-head ones col (block col 0)
    ones_in = np.ones((128, 128), np.float32)
    return _round_fp32r(xT_all), masks, vinit, ones_in, per_group


def kernel(x, w_qkv, w_out):
    x = np.asarray(x)
    w_qkv = np.asarray(w_qkv)
    w_out = np.asarray(w_out)
    xT_all, masks, vinit, ones_in, per_group = _host_prep(x, w_qkv, w_out)

    if not _NC_CACHE:
        _NC_CACHE.append(_build_nc())
    nc = _NC_CACHE[0]

    in_maps = []
    for core in range(8):
        b, g = core // 2, core % 2
        wqk_r, wv_t, wo_t = per_group[g]
        in_maps.append(
            {"xT": xT_all[b], "wqk": wqk_r, "wv": wv_t, "wo": wo_t, "masks": masks,
             "vinit": vinit, "ones_in": ones_in}
        )

    res = bass_utils.run_bass_kernel_spmd(
        nc, in_maps, core_ids=list(range(8)), trace=TRACE
    )
    LAST_RUN["res"] = res

    y = np.empty((B, T, C), np.float32)
    for b in range(B):
        y[b] = res.results[2 * b]["y"] + res.results[2 * b + 1]["y"]
    return y
